# revision 33
# baseline (speedup 1.0000x reference)
"""Trainium2 Bass kernel for nn_DGASEncoder (PointNet++-style ball-query encoder).

Self-contained: hardcoded shapes; takes FULL inputs, shards across 8 NeuronCores
(data-parallel over (batch, N/2)), returns the FULL output.

Per-core pipeline (SPMD; cores differ only in input data):
  A) conv1d+BN stats: h = W1 @ f over all B*N via PE; bn_stats/bn_aggr -> mean/var;
     h_n = relu(gamma'*h_b + beta') for this core's batch.
  B) ball query: squared distances via a 24-row bf16-split matmul (f32-exact dot);
     s = sign(r^2-d2) on ACT; neighbor ranks via triangular matmuls + carry matmuls;
     slot ids (32-R, invalid -> negative) evacuated as int16.
  C) first-K extraction with gpsimd local_scatter (all 16 tiles into one
     dst_all tile; single pad-fix pass doubles as the scatter->gather ordering
     barrier so the gpsimd library only reloads twice); one 4096-descriptor
     dma_gather per query tile from the bf16 hT table; bf16 add + max-pool
     over K against the (host k-reversed, bf16) dlp stream.
"""
import functools
import numpy as np
import ml_dtypes

B, N, C, K = 4, 4096, 128, 32
RADIUS = 0.1
BN_EPS = 1e-5
R2 = RADIUS * RADIUS
BIG = 64.0
NCHUNK = 128
NCH = N // NCHUNK          # 32
QPC = N // 2               # 2048 queries per core
QT = 128                   # queries per tile
NQT = QPC // QT            # 16
QB = 512                   # query block
NQB = QPC // QB            # 4
NCORES = 8

bf16 = ml_dtypes.bfloat16


def _bf(x):
    return np.asarray(x, dtype=bf16).astype(np.float32)


def _split3(x):
    h = _bf(x)
    m = _bf(x - h)
    l = _bf(x - h - m)
    return h, m, l


def _build_AB(p_b, q_lo, q_hi):
    """A [24, N] (candidate side, -2 scale folded), Bm [24, Q] (query side), bf16."""
    x = p_b.astype(np.float32)
    pn = np.sum(x * x, axis=1, dtype=np.float32)
    sp = [_split3(x[:, d]) for d in range(3)]
    sp2 = [tuple(-2.0 * t for t in sp[d]) for d in range(3)]
    spn = _split3(pn)
    PAIRS = [(0, 0), (0, 1), (1, 0), (0, 2), (2, 0), (1, 1)]
    A_rows, B_rows = [], []
    for d in range(3):
        for (s, s2) in PAIRS:
            A_rows.append(sp2[d][s])
            B_rows.append(sp[d][s2][q_lo:q_hi])
    for s in range(3):
        A_rows.append(spn[s])
        B_rows.append(np.ones(q_hi - q_lo, np.float32))
    for s in range(3):
        A_rows.append(np.ones(N, np.float32))
        B_rows.append(spn[s][q_lo:q_hi])
    return (np.stack(A_rows).astype(bf16), np.stack(B_rows).astype(bf16))


def _static_tables():
    # tri_ext for s01 in {0,1}: col l has -1 above the diagonal (counts valid
    # j<l), +96 on it (validity kicker); col 128 is all-ones (chunk count).
    # slot = (tri-matmul) + (-pref - 65): valid -> 31 - rank, invalid <= -65.
    W = np.zeros((NCHUNK, NCHUNK + 4), np.float32)
    for jl in range(NCHUNK):
        W[:jl, jl] = -1.0
        W[jl, jl] = 96.0
    W[:, NCHUNK] = 1.0
    iota_j = np.broadcast_to(np.arange(N, dtype=np.int16)[None, :], (128, N)).copy()
    ident = np.eye(128, dtype=np.float32)
    return W.astype(bf16), iota_j, ident


def _kernel_body(tc, io):
    import concourse.bass as bass
    import concourse.mybir as mybir
    from contextlib import ExitStack

    nc = tc.nc
    dt = mybir.dt
    AF = mybir.ActivationFunctionType
    ALU = mybir.AluOpType

    with ExitStack() as ctx:
        const = ctx.enter_context(tc.tile_pool(name="const", bufs=1))
        w1t_sb = const.tile([128, 128], dt.bfloat16, tag="w1t")
        nc.sync.dma_start(out=w1t_sb, in_=io["w1t"])
        gamma_sb = const.tile([128, 1], dt.float32, tag="gm")
        nc.sync.dma_start(out=gamma_sb, in_=io["gamma"])
        beta_sb = const.tile([128, 1], dt.float32, tag="bt")
        nc.sync.dma_start(out=beta_sb, in_=io["beta"])
        pA_sb = const.tile([24, N], dt.bfloat16, tag="pA")
        nc.sync.dma_start(out=pA_sb, in_=io["pA"])
        pB_sb = const.tile([24, QPC], dt.bfloat16, tag="pB")
        nc.sync.dma_start(out=pB_sb, in_=io["pB"])
        tri_sb = const.tile([128, NCHUNK + 4], dt.bfloat16, tag="tri")
        nc.sync.dma_start(out=tri_sb, in_=io["tri_w"])
        iota_sb = const.tile([128, N], dt.int16, tag="iota")
        nc.sync.dma_start(out=iota_sb, in_=io["iota_j"])
        ident_sb = const.tile([128, 128], dt.float32, tag="idn")
        nc.sync.dma_start(out=ident_sb, in_=io["ident"])
        eps_t = const.tile([128, 1], dt.float32, tag="eps")
        nc.vector.memset(eps_t, BN_EPS)

        hpool = ctx.enter_context(tc.tile_pool(name="hp", bufs=1))
        h_n = hpool.tile([128, N], dt.float32, tag="hn")
        hT = hpool.tile([128, N], dt.bfloat16, tag="hT")
        dram = ctx.enter_context(tc.tile_pool(name="dram", bufs=1, space="DRAM"))
        hT_d = dram.tile([N, 128], dt.bfloat16, tag="hTd")

        # ---------------- Phase A: BN stats + h_n ----------------
        with ExitStack() as actx:
            bigf = actx.enter_context(tc.tile_pool(name="bigf", bufs=1))
            ps_h = actx.enter_context(tc.tile_pool(name="ps_h", bufs=4, space="PSUM"))
            stp = actx.enter_context(tc.tile_pool(name="stp", bufs=1))

            # f_all is batch-rotated on host so this core's batch occupies cols [0, N)
            f_sb = bigf.tile([128, B * N], dt.bfloat16, tag="f")
            nc.sync.dma_start(out=f_sb, in_=io["f_all"])
            f_b = f_sb[:, 0:N]

            stats = stp.tile([128, B * N // 512, 6], dt.float32, tag="st")
            for i in range(B * N // 512):
                ph = ps_h.tile([128, 512], dt.float32, tag="ph")
                nc.tensor.matmul(ph, w1t_sb, f_sb[:, i * 512:(i + 1) * 512],
                                 start=True, stop=True)
                nc.vector.bn_stats(out=stats[:, i, :], in_=ph)
            mv = stp.tile([128, 2], dt.float32, tag="mv")
            nc.vector.bn_aggr(out=mv, in_=stats)
            # rstd = 1/sqrt(var+eps); gamma2 = gamma*rstd; bias2 = beta - gamma2*mean
            sqv = stp.tile([128, 1], dt.float32, tag="sq")
            nc.scalar.activation(sqv, mv[:, 1:2], AF.Sqrt, bias=eps_t, scale=1.0)
            rstd = stp.tile([128, 1], dt.float32, tag="rs")
            nc.vector.reciprocal(rstd, sqv)
            gamma2 = stp.tile([128, 1], dt.float32, tag="g2")
            nc.vector.tensor_mul(gamma2, gamma_sb, rstd)
            gm = stp.tile([128, 1], dt.float32, tag="gmn")
            nc.vector.tensor_mul(gm, gamma2, mv[:, 0:1])
            bias2 = stp.tile([128, 1], dt.float32, tag="b2")
            nc.vector.tensor_sub(bias2, beta_sb, gm)

            for i in range(N // 512):
                ph = ps_h.tile([128, 512], dt.float32, tag="ph")
                nc.tensor.matmul(ph, w1t_sb, f_b[:, i * 512:(i + 1) * 512],
                                 start=True, stop=True)
                nc.scalar.activation(h_n[:, i * 512:(i + 1) * 512], ph, AF.Relu,
                                     bias=bias2, scale=gamma2)

        # hT fp16 rows in DRAM for the dma_gather: row n = h_n[:, n]
        with ExitStack() as tctx:
            ps_t = tctx.enter_context(tc.tile_pool(name="ps_t", bufs=2, space="PSUM"))
            for c2 in range(N // 128):
                tp2 = ps_t.tile([128, 128], dt.float32, tag="htp")
                nc.tensor.transpose(tp2, h_n[:, c2 * 128:(c2 + 1) * 128], ident_sb)
                nc.scalar.activation(hT[:, c2 * 128:(c2 + 1) * 128], tp2, AF.Copy)
            # hT sbuf [tok%128, (tok//128)*128ch] -> DRAM rows [tok, ch]
            hT_v = hT.rearrange("p (s c) -> p s c", c=128)
            hTd_v = bass.AP(tensor=hT_d.tensor, offset=hT_d.offset,
                            ap=[[128, 128], [16384, N // 128], [1, 128]])
            nc.sync.dma_start(out=hTd_v, in_=hT_v)

        # ---------------- Phase B+C ----------------
        spool = ctx.enter_context(tc.tile_pool(name="sp", bufs=1))
        ps_sq = ctx.enter_context(tc.tile_pool(name="ps_sq", bufs=2, space="PSUM"))
        ps_tr = ctx.enter_context(tc.tile_pool(name="ps_tr", bufs=1, space="PSUM"))
        ps_rk = ctx.enter_context(tc.tile_pool(name="ps_rk", bufs=2, space="PSUM"))
        small = ctx.enter_context(tc.tile_pool(name="small", bufs=3))
        ipool = ctx.enter_context(tc.tile_pool(name="ip", bufs=2))
        iwpool = ctx.enter_context(tc.tile_pool(name="iw", bufs=1))
        fpool = ctx.enter_context(tc.tile_pool(name="fp", bufs=2))
        dpool = ctx.enter_context(tc.tile_pool(name="dp", bufs=2))
        opool = ctx.enter_context(tc.tile_pool(name="op", bufs=3))

        dlp_v = io["dlp_s"]          # [128, QPC*K] bf16, k reversed on host
        out_v = io["out_o"]          # [128, QPC]

        dstp = ctx.enter_context(tc.tile_pool(name="dstp", bufs=1))
        dst_all = dstp.tile([128, NQT * K], dt.int16, tag="dsta")

        for qb in range(NQB):
            qbs = slice(qb * QB, (qb + 1) * QB)
            s_t = []
            for c in range(NCH):
                psq = ps_sq.tile([128, QB], dt.float32, tag="sq")
                nc.tensor.matmul(psq, pA_sb[:, c * NCHUNK:(c + 1) * NCHUNK],
                                 pB_sb[:, qbs], start=True, stop=True)
                st = spool.tile([128, QB], dt.bfloat16, tag=f"s{c}")
                # s01 = 1.0 if d2 <= r2 else 0.0
                nc.vector.tensor_scalar(st, psq, R2, None, op0=ALU.is_le)
                s_t.append(st)

            for qt in range(NQB):
                qs = slice(qt * QT, (qt + 1) * QT)
                ti = qb * NQB + qt

                # rank matmuls: 129 cols per chunk (128 uncarried slot ids +
                # the chunk's valid count); 3 chunks per PSUM bank
                idxu = ipool.tile([128, N], dt.bfloat16, tag="iu")
                msb = small.tile([128, NCH], dt.float32, tag="msb")
                for c0 in range(0, NCH, 3):
                    gn = min(3, NCH - c0)
                    pr = ps_rk.tile([128, 512], dt.float32, tag="rk")
                    for cc in range(gn):
                        nc.tensor.matmul(pr[:, cc * 129:cc * 129 + 129],
                                         s_t[c0 + cc][:, qs], tri_sb[:, :129],
                                         start=True, stop=True)
                    s0 = pr[:, 128:129]
                    nc.vector.tensor_copy(
                        msb[:, c0:c0 + gn],
                        bass.AP(tensor=s0.tensor, offset=s0.offset,
                                ap=[s0.ap[0], [129, gn]]))
                    sl0 = pr[:, 0:128]
                    nc.scalar.activation(
                        idxu[:, c0 * 128:(c0 + gn) * 128].rearrange(
                            "p (c l) -> p c l", l=128),
                        bass.AP(tensor=sl0.tensor, offset=sl0.offset,
                                ap=[sl0.ap[0], [129, gn], [1, 128]]),
                        AF.Copy)

                # exclusive prefix of counts -> carry = -pref - 65
                pref = small.tile([128, NCH], dt.float32, tag="pf0")
                nc.vector.memset(pref[:, 0:1], 0.0)
                nc.vector.tensor_copy(pref[:, 1:], msb[:, :NCH - 1])
                for sh in (1, 2, 4, 8, 16):
                    pref2 = small.tile([128, NCH], dt.float32, tag=f"pf{sh}")
                    nc.vector.tensor_copy(pref2[:, :sh], pref[:, :sh])
                    nc.vector.tensor_add(pref2[:, sh:], pref[:, sh:], pref[:, :NCH - sh])
                    pref = pref2
                carry = small.tile([128, NCH], dt.float32, tag="cry")
                nc.vector.tensor_scalar(carry, pref, -1.0, -65.0,
                                        op0=ALU.mult, op1=ALU.add)

                # final slot ids = idxu + carry (broadcast over each chunk)
                idxs_sc = ipool.tile([128, N], dt.int16, tag="isc")
                nc.vector.tensor_add(
                    idxs_sc.rearrange("p (c l) -> p c l", l=128),
                    idxu.rearrange("p (c l) -> p c l", l=128),
                    bass.AP(tensor=carry.tensor, offset=carry.offset,
                            ap=[carry.ap[0], [1, NCH], [0, 128]]))

                # first-K extraction into this tile's slice of dst_all
                nc.gpsimd.local_scatter(dst_all[:, ti * K:(ti + 1) * K],
                                        iota_sb, idxs_sc,
                                        channels=128, num_elems=K, num_idxs=N)

        # pad-slot fix over ALL tiles at once. Reading the whole dst_all makes
        # this (and everything downstream, incl. every dma_gather) depend on
        # every local_scatter -> exactly one gpsimd library reload each way.
        mask_all = small.tile([128, NQT * K], dt.int8, tag="mska")
        nc.vector.tensor_scalar(mask_all, dst_all, 0.0, None, op0=ALU.is_equal)
        for ti in range(NQT):
            dsl = dst_all[:, ti * K:(ti + 1) * K]
            nc.vector.copy_predicated(
                dsl, mask_all[:, ti * K:(ti + 1) * K],
                dst_all[:, ti * K + K - 1:ti * K + K].to_broadcast((128, K)))

        # replicate x8 and transpose into wrapped dma_gather index layout
        # layout (h, r, s16): half h cols contiguous for the transpose
        idxw_all = iwpool.tile([128, NQT * 256], dt.int16, tag="idxwa")
        for ti in range(NQT):
            dstf = small.tile([128, 256], dt.float32, tag="dstf")
            dfv = dstf.rearrange("p (h r s) -> p h r s", h=2, r=8)
            sl = dst_all[:, ti * K:(ti + 1) * K]
            dst_b = bass.AP(tensor=sl.tensor, offset=sl.offset,
                            ap=[sl.ap[0], [16, 2], [0, 8], [1, 16]])
            nc.vector.tensor_copy(dfv, dst_b)
            iwv = idxw_all[:, ti * 256:(ti + 1) * 256].rearrange(
                "p (c two) -> p c two", two=2)
            for half in range(2):
                tps = ps_tr.tile([128, 128], dt.float32, tag="tp")
                nc.tensor.transpose(tps, dstf[:, half * 128:(half + 1) * 128],
                                    ident_sb)
                nc.scalar.activation(iwv[:, :, half], tps, AF.Copy)

        # ---------------- Loop C: gathers + pooling ----------------
        for ti in range(NQT):
            fj = fpool.tile([128, QT * K], dt.bfloat16, tag="fj")
            fj_v = fj.rearrange("p (o i) -> p o i", o=1)
            GC = 512   # per-ring cap on HW; pipeline calls across 4 SWDGE queues
            for gc in range(QT * K // GC):
                nc.gpsimd.dma_gather(
                    fj_v[:, :, gc * GC:(gc + 1) * GC], hT_d,
                    idxw_all[:, ti * 256 + gc * (GC // 16):
                             ti * 256 + (gc + 1) * (GC // 16)],
                    num_idxs=GC, num_idxs_reg=GC, elem_size=128,
                    transpose=True, queue_num=(ti * (QT * K // GC) + gc) % 2)
            dlp_t = dpool.tile([128, QT * K], dt.bfloat16, tag="dl")
            nc.sync.dma_start(out=dlp_t, in_=dlp_v[:, ti * QT * K:(ti + 1) * QT * K])
            nc.vector.tensor_add(fj, fj, dlp_t)
            out_t = opool.tile([128, QT], dt.float32, tag="ot")
            nc.vector.tensor_reduce(out_t,
                                    fj.rearrange("p (q k) -> p q k", k=K),
                                    axis=mybir.AxisListType.X, op=ALU.max)
            nc.sync.dma_start(out=out_v[:, ti * QT:(ti + 1) * QT], in_=out_t)


@functools.lru_cache(maxsize=1)
def _compiled():
    import concourse.bass as bass
    import concourse.bacc as bacc
    import concourse.tile as tile
    import concourse.mybir as mybir

    dt = mybir.dt
    nc = bacc.Bacc("TRN2", target_bir_lowering=False, debug=False,
                   num_devices=NCORES, dynamic_dma_scratch_size=32768,
                   num_swdge_queues=2)
    io = {}

    def din(name, shape, dtype):
        io[name] = nc.dram_tensor(name, shape, dtype, kind="ExternalInput").ap()

    din("f_all", [128, B * N], dt.bfloat16)
    din("w1t", [128, 128], dt.bfloat16)
    din("gamma", [128, 1], dt.float32)
    din("beta", [128, 1], dt.float32)
    din("pA", [24, N], dt.bfloat16)
    din("pB", [24, QPC], dt.bfloat16)
    din("tri_w", [128, NCHUNK + 4], dt.bfloat16)
    din("iota_j", [128, N], dt.int16)
    din("ident", [128, 128], dt.float32)
    din("dlp_s", [128, QPC * K], dt.bfloat16)
    io["out_o"] = nc.dram_tensor("out_o", [128, QPC], dt.float32,
                                 kind="ExternalOutput").ap()

    with tile.TileContext(nc) as tc:
        _kernel_body(tc, io)
    nc.compile()
    return nc


def _host_prep(inputs):
    p = np.asarray(inputs["p"], np.float32)
    f = np.asarray(inputs["f"], np.float32)
    dlp = np.asarray(inputs["dlp"], np.float32)
    W1 = np.asarray(inputs["W1"], np.float32)
    gamma = np.asarray(inputs["gamma"], np.float32)
    beta = np.asarray(inputs["beta"], np.float32)

    tri_w, iota_j, ident = _static_tables()
    f_cbn = np.moveaxis(f, 0, 1)                       # [C, B, N]
    w1t = np.ascontiguousarray(W1.T).astype(bf16)

    in_maps = []
    for core in range(NCORES):
        b, half = core // 2, core % 2
        q_lo, q_hi = half * QPC, (half + 1) * QPC
        pA, pB = _build_AB(p[b], q_lo, q_hi)
        dlp_s = np.ascontiguousarray(
            dlp[b, :, q_lo:q_hi, ::-1].reshape(C, QPC * K)).astype(bf16)
        # batch-rotated so this core's batch b sits in cols [0, N)
        f_all = np.ascontiguousarray(
            np.roll(f_cbn, -b, axis=1).reshape(C, B * N)).astype(bf16)
        in_maps.append({
            "f_all": f_all,
            "w1t": w1t,
            "gamma": np.ascontiguousarray(gamma.reshape(C, 1)),
            "beta": np.ascontiguousarray(beta.reshape(C, 1)),
            "pA": pA, "pB": pB,
            "tri_w": tri_w, "iota_j": iota_j, "ident": ident,
            "dlp_s": dlp_s,
        })
    return in_maps


def run(inputs, trace=False, **kw):
    from concourse.bass_utils import run_bass_kernel_spmd
    nc = _compiled()
    in_maps = _host_prep(inputs)
    res = run_bass_kernel_spmd(nc, in_maps, core_ids=list(range(NCORES)),
                               trace=trace, **kw)
    out = np.zeros((B, C, N), np.float32)
    for core in range(NCORES):
        b, half = core // 2, core % 2
        out[b, :, half * QPC:(half + 1) * QPC] = res.results[core]["out_o"]
    return out, res


def kernel(**inputs) -> np.ndarray:
    out, _ = run(inputs, trace=False)
    return out



# revision 37
# speedup vs baseline: 1.0307x; 1.0307x over previous
"""Trainium2 Bass kernel for nn_DGASEncoder (PointNet++-style ball-query encoder).

Self-contained: hardcoded shapes; takes FULL inputs, shards across 8 NeuronCores
(data-parallel over (batch, N/2)), returns the FULL output.

Per-core pipeline (SPMD; cores differ only in input data):
  A) conv1d+BN stats: h = W1 @ f over all B*N via PE; bn_stats/bn_aggr -> mean/var;
     h_n = relu(gamma'*h_b + beta') for this core's batch.
  B) ball query: squared distances via a 24-row bf16-split matmul (f32-exact dot);
     s = sign(r^2-d2) on ACT; neighbor ranks via triangular matmuls + carry matmuls;
     slot ids (32-R, invalid -> negative) evacuated as int16.
  C) first-K extraction with gpsimd local_scatter (all 16 tiles into one
     dst_all tile; single pad-fix pass doubles as the scatter->gather ordering
     barrier so the gpsimd library only reloads twice); one 4096-descriptor
     dma_gather per query tile from the bf16 hT table; bf16 add + max-pool
     over K against the (host k-reversed, bf16) dlp stream.
"""
import functools
import numpy as np
import ml_dtypes

B, N, C, K = 4, 4096, 128, 32
RADIUS = 0.1
BN_EPS = 1e-5
R2 = RADIUS * RADIUS
BIG = 64.0
NCHUNK = 128
NCH = N // NCHUNK          # 32
QPC = N // 2               # 2048 queries per core
QT = 128                   # queries per tile
NQT = QPC // QT            # 16
QB = 512                   # query block
NQB = QPC // QB            # 4
NCORES = 8

bf16 = ml_dtypes.bfloat16


def _bf(x):
    return np.asarray(x, dtype=bf16).astype(np.float32)


def _split3(x):
    h = _bf(x)
    m = _bf(x - h)
    l = _bf(x - h - m)
    return h, m, l


def _build_AB(p_b, q_lo, q_hi):
    """A [24, N] (candidate side, -2 scale folded), Bm [24, Q] (query side), bf16."""
    x = p_b.astype(np.float32)
    pn = np.sum(x * x, axis=1, dtype=np.float32)
    sp = [_split3(x[:, d]) for d in range(3)]
    sp2 = [tuple(-2.0 * t for t in sp[d]) for d in range(3)]
    spn = _split3(pn)
    PAIRS = [(0, 0), (0, 1), (1, 0), (0, 2), (2, 0), (1, 1)]
    A_rows, B_rows = [], []
    for d in range(3):
        for (s, s2) in PAIRS:
            A_rows.append(sp2[d][s])
            B_rows.append(sp[d][s2][q_lo:q_hi])
    for s in range(3):
        A_rows.append(spn[s])
        B_rows.append(np.ones(q_hi - q_lo, np.float32))
    for s in range(3):
        A_rows.append(np.ones(N, np.float32))
        B_rows.append(spn[s][q_lo:q_hi])
    return (np.stack(A_rows).astype(bf16), np.stack(B_rows).astype(bf16))


def _static_tables():
    # tri_ext for s01 in {0,1}: col l has -1 above the diagonal (counts valid
    # j<l), +96 on it (validity kicker); col 128 is all-ones (chunk count).
    # slot = (tri-matmul) + (-pref - 65): valid -> 31 - rank, invalid <= -65.
    W = np.zeros((NCHUNK, NCHUNK + 4), np.float32)
    for jl in range(NCHUNK):
        W[:jl, jl] = -1.0
        W[jl, jl] = 96.0
    W[:, NCHUNK] = 1.0
    iota_j = np.broadcast_to(np.arange(N, dtype=np.int16)[None, :], (128, N)).copy()
    ident = np.eye(128, dtype=np.float32)
    return W.astype(bf16), iota_j, ident


def _kernel_body(tc, io):
    import concourse.bass as bass
    import concourse.mybir as mybir
    from contextlib import ExitStack

    nc = tc.nc
    dt = mybir.dt
    AF = mybir.ActivationFunctionType
    ALU = mybir.AluOpType

    with ExitStack() as ctx:
        const = ctx.enter_context(tc.tile_pool(name="const", bufs=1))
        w1t_sb = const.tile([128, 128], dt.bfloat16, tag="w1t")
        nc.sync.dma_start(out=w1t_sb, in_=io["w1t"])
        gamma_sb = const.tile([128, 1], dt.float32, tag="gm")
        nc.sync.dma_start(out=gamma_sb, in_=io["gamma"])
        beta_sb = const.tile([128, 1], dt.float32, tag="bt")
        nc.sync.dma_start(out=beta_sb, in_=io["beta"])
        pA_sb = const.tile([24, N], dt.bfloat16, tag="pA")
        nc.sync.dma_start(out=pA_sb, in_=io["pA"])
        pB_sb = const.tile([24, QPC], dt.bfloat16, tag="pB")
        nc.sync.dma_start(out=pB_sb, in_=io["pB"])
        tri_sb = const.tile([128, NCHUNK + 4], dt.bfloat16, tag="tri")
        nc.sync.dma_start(out=tri_sb, in_=io["tri_w"])
        iota_sb = const.tile([128, N], dt.int16, tag="iota")
        nc.sync.dma_start(out=iota_sb, in_=io["iota_j"])
        ident_sb = const.tile([128, 128], dt.float32, tag="idn")
        nc.sync.dma_start(out=ident_sb, in_=io["ident"])
        eps_t = const.tile([128, 1], dt.float32, tag="eps")
        nc.vector.memset(eps_t, BN_EPS)
        zeros31 = const.tile([128, NCH - 1], dt.float32, tag="z31")
        nc.vector.memset(zeros31, 0.0)

        hpool = ctx.enter_context(tc.tile_pool(name="hp", bufs=1))
        h_n = hpool.tile([128, N], dt.float32, tag="hn")
        hT = hpool.tile([128, N], dt.bfloat16, tag="hT")
        dram = ctx.enter_context(tc.tile_pool(name="dram", bufs=1, space="DRAM"))
        hT_d = dram.tile([N, 128], dt.bfloat16, tag="hTd")

        # ---------------- Phase A: BN stats + h_n ----------------
        with ExitStack() as actx:
            bigf = actx.enter_context(tc.tile_pool(name="bigf", bufs=1))
            ps_h = actx.enter_context(tc.tile_pool(name="ps_h", bufs=4, space="PSUM"))
            stp = actx.enter_context(tc.tile_pool(name="stp", bufs=1))

            # f_all is batch-rotated on host so this core's batch occupies cols [0, N)
            f_sb = bigf.tile([128, B * N], dt.bfloat16, tag="f")
            nc.sync.dma_start(out=f_sb, in_=io["f_all"])
            f_b = f_sb[:, 0:N]

            stats = stp.tile([128, B * N // 512, 6], dt.float32, tag="st")
            for i in range(B * N // 512):
                ph = ps_h.tile([128, 512], dt.float32, tag="ph")
                nc.tensor.matmul(ph, w1t_sb, f_sb[:, i * 512:(i + 1) * 512],
                                 start=True, stop=True)
                nc.vector.bn_stats(out=stats[:, i, :], in_=ph)
            mv = stp.tile([128, 2], dt.float32, tag="mv")
            nc.vector.bn_aggr(out=mv, in_=stats)
            # rstd = 1/sqrt(var+eps); gamma2 = gamma*rstd; bias2 = beta - gamma2*mean
            sqv = stp.tile([128, 1], dt.float32, tag="sq")
            nc.scalar.activation(sqv, mv[:, 1:2], AF.Sqrt, bias=eps_t, scale=1.0)
            rstd = stp.tile([128, 1], dt.float32, tag="rs")
            nc.vector.reciprocal(rstd, sqv)
            gamma2 = stp.tile([128, 1], dt.float32, tag="g2")
            nc.vector.tensor_mul(gamma2, gamma_sb, rstd)
            gm = stp.tile([128, 1], dt.float32, tag="gmn")
            nc.vector.tensor_mul(gm, gamma2, mv[:, 0:1])
            bias2 = stp.tile([128, 1], dt.float32, tag="b2")
            nc.vector.tensor_sub(bias2, beta_sb, gm)

            for i in range(N // 512):
                ph = ps_h.tile([128, 512], dt.float32, tag="ph")
                nc.tensor.matmul(ph, w1t_sb, f_b[:, i * 512:(i + 1) * 512],
                                 start=True, stop=True)
                nc.scalar.activation(h_n[:, i * 512:(i + 1) * 512], ph, AF.Relu,
                                     bias=bias2, scale=gamma2)

        # hT fp16 rows in DRAM for the dma_gather: row n = h_n[:, n]
        with ExitStack() as tctx:
            ps_t = tctx.enter_context(tc.tile_pool(name="ps_t", bufs=2, space="PSUM"))
            for c2 in range(N // 128):
                tp2 = ps_t.tile([128, 128], dt.float32, tag="htp")
                nc.tensor.transpose(tp2, h_n[:, c2 * 128:(c2 + 1) * 128], ident_sb)
                nc.scalar.activation(hT[:, c2 * 128:(c2 + 1) * 128], tp2, AF.Copy)
            # hT sbuf [tok%128, (tok//128)*128ch] -> DRAM rows [tok, ch]
            hT_v = hT.rearrange("p (s c) -> p s c", c=128)
            hTd_v = bass.AP(tensor=hT_d.tensor, offset=hT_d.offset,
                            ap=[[128, 128], [16384, N // 128], [1, 128]])
            nc.sync.dma_start(out=hTd_v, in_=hT_v)

        # ---------------- Phase B+C ----------------
        spool = ctx.enter_context(tc.tile_pool(name="sp", bufs=1))
        ps_sq = ctx.enter_context(tc.tile_pool(name="ps_sq", bufs=2, space="PSUM"))
        ps_tr = ctx.enter_context(tc.tile_pool(name="ps_tr", bufs=1, space="PSUM"))
        ps_rk = ctx.enter_context(tc.tile_pool(name="ps_rk", bufs=2, space="PSUM"))
        small = ctx.enter_context(tc.tile_pool(name="small", bufs=3))
        ipool = ctx.enter_context(tc.tile_pool(name="ip", bufs=2))
        iwpool = ctx.enter_context(tc.tile_pool(name="iw", bufs=1))
        fpool = ctx.enter_context(tc.tile_pool(name="fp", bufs=2))
        dpool = ctx.enter_context(tc.tile_pool(name="dp", bufs=2))
        opool = ctx.enter_context(tc.tile_pool(name="op", bufs=3))

        dlp_v = io["dlp_s"]          # [128, QPC*K] bf16, k reversed on host
        out_v = io["out_o"]          # [128, QPC]

        dstp = ctx.enter_context(tc.tile_pool(name="dstp", bufs=1))
        dst_all = dstp.tile([128, NQT * K], dt.int16, tag="dsta")

        for qb in range(NQB):
            qbs = slice(qb * QB, (qb + 1) * QB)
            s_t = []
            for c in range(NCH):
                psq = ps_sq.tile([128, QB], dt.float32, tag="sq")
                nc.tensor.matmul(psq, pA_sb[:, c * NCHUNK:(c + 1) * NCHUNK],
                                 pB_sb[:, qbs], start=True, stop=True)
                st = spool.tile([128, QB], dt.bfloat16, tag=f"s{c}")
                # s01 = 1.0 if d2 <= r2 else 0.0
                nc.vector.tensor_scalar(st, psq, R2, None, op0=ALU.is_le)
                s_t.append(st)

            for qt in range(NQB):
                qs = slice(qt * QT, (qt + 1) * QT)
                ti = qb * NQB + qt

                # rank matmuls: 129 cols per chunk (128 uncarried slot ids +
                # the chunk's valid count); 3 chunks per PSUM bank
                idxu = ipool.tile([128, N], dt.bfloat16, tag="iu")
                msb = small.tile([128, NCH], dt.float32, tag="msb")
                for c0 in range(0, NCH, 3):
                    gn = min(3, NCH - c0)
                    pr = ps_rk.tile([128, 512], dt.float32, tag="rk")
                    for cc in range(gn):
                        nc.tensor.matmul(pr[:, cc * 129:cc * 129 + 129],
                                         s_t[c0 + cc][:, qs], tri_sb[:, :129],
                                         start=True, stop=True)
                    s0 = pr[:, 128:129]
                    # negated count so the scan below yields the carry directly
                    nc.vector.tensor_scalar(
                        msb[:, c0:c0 + gn],
                        bass.AP(tensor=s0.tensor, offset=s0.offset,
                                ap=[s0.ap[0], [129, gn]]),
                        -1.0, None, op0=ALU.mult)
                    sl0 = pr[:, 0:128]
                    nc.scalar.activation(
                        idxu[:, c0 * 128:(c0 + gn) * 128].rearrange(
                            "p (c l) -> p c l", l=128),
                        bass.AP(tensor=sl0.tensor, offset=sl0.offset,
                                ap=[sl0.ap[0], [129, gn], [1, 128]]),
                        AF.Copy)

                # carry[q,c] = -65 - sum_{c'<c} cnt  (exclusive scan of -cnt).
                # |carry| <= 65 + V(q) stays well under 256, so bf16 is exact.
                carry = small.tile([128, NCH], dt.bfloat16, tag="cry")
                nc.vector.memset(carry[:, 0:1], -65.0)
                nc.vector.tensor_tensor_scan(
                    carry[:, 1:], msb[:, :NCH - 1], zeros31, -65.0,
                    op0=ALU.add, op1=ALU.add)

                # final slot ids = idxu + carry (broadcast over each chunk)
                idxs_sc = ipool.tile([128, N], dt.int16, tag="isc")
                nc.vector.tensor_add(
                    idxs_sc.rearrange("p (c l) -> p c l", l=128),
                    idxu.rearrange("p (c l) -> p c l", l=128),
                    bass.AP(tensor=carry.tensor, offset=carry.offset,
                            ap=[carry.ap[0], [1, NCH], [0, 128]]))

                # first-K extraction into this tile's slice of dst_all
                nc.gpsimd.local_scatter(dst_all[:, ti * K:(ti + 1) * K],
                                        iota_sb, idxs_sc,
                                        channels=128, num_elems=K, num_idxs=N)

        # pad-slot fix over ALL tiles at once. Reading the whole dst_all makes
        # this (and everything downstream, incl. every dma_gather) depend on
        # every local_scatter -> exactly one gpsimd library reload each way.
        mask_all = small.tile([128, NQT * K], dt.int8, tag="mska")
        nc.vector.tensor_scalar(mask_all, dst_all, 0.0, None, op0=ALU.is_equal)
        for ti in range(NQT):
            dsl = dst_all[:, ti * K:(ti + 1) * K]
            nc.vector.copy_predicated(
                dsl, mask_all[:, ti * K:(ti + 1) * K],
                dst_all[:, ti * K + K - 1:ti * K + K].to_broadcast((128, K)))

        # replicate x8 and transpose into wrapped dma_gather index layout
        # layout (h, r, s16): half h cols contiguous for the transpose
        idxw_all = iwpool.tile([128, NQT * 256], dt.int16, tag="idxwa")
        for ti in range(NQT):
            dstf = small.tile([128, 256], dt.float32, tag="dstf")
            dfv = dstf.rearrange("p (h r s) -> p h r s", h=2, r=8)
            sl = dst_all[:, ti * K:(ti + 1) * K]
            dst_b = bass.AP(tensor=sl.tensor, offset=sl.offset,
                            ap=[sl.ap[0], [16, 2], [0, 8], [1, 16]])
            nc.vector.tensor_copy(dfv, dst_b)
            iwv = idxw_all[:, ti * 256:(ti + 1) * 256].rearrange(
                "p (c two) -> p c two", two=2)
            for half in range(2):
                tps = ps_tr.tile([128, 128], dt.float32, tag="tp")
                nc.tensor.transpose(tps, dstf[:, half * 128:(half + 1) * 128],
                                    ident_sb)
                nc.scalar.activation(iwv[:, :, half], tps, AF.Copy)

        # ---------------- Loop C: gathers + pooling ----------------
        for ti in range(NQT):
            fj = fpool.tile([128, QT * K], dt.bfloat16, tag="fj")
            fj_v = fj.rearrange("p (o i) -> p o i", o=1)
            GC = 512   # per-ring cap on HW; pipeline calls across 4 SWDGE queues
            for gc in range(QT * K // GC):
                nc.gpsimd.dma_gather(
                    fj_v[:, :, gc * GC:(gc + 1) * GC], hT_d,
                    idxw_all[:, ti * 256 + gc * (GC // 16):
                             ti * 256 + (gc + 1) * (GC // 16)],
                    num_idxs=GC, num_idxs_reg=GC, elem_size=128,
                    transpose=True, queue_num=(ti * (QT * K // GC) + gc) % 2)
            dlp_t = dpool.tile([128, QT * K], dt.bfloat16, tag="dl")
            nc.sync.dma_start(out=dlp_t, in_=dlp_v[:, ti * QT * K:(ti + 1) * QT * K])
            nc.vector.tensor_add(fj, fj, dlp_t)
            out_b = opool.tile([128, QT], dt.bfloat16, tag="ob")
            nc.vector.tensor_reduce(out_b,
                                    fj.rearrange("p (q k) -> p q k", k=K),
                                    axis=mybir.AxisListType.X, op=ALU.max)
            out_t = opool.tile([128, QT], dt.float32, tag="ot")
            nc.scalar.activation(out_t, out_b, AF.Copy)
            nc.sync.dma_start(out=out_v[:, ti * QT:(ti + 1) * QT], in_=out_t)


@functools.lru_cache(maxsize=1)
def _compiled():
    import concourse.bass as bass
    import concourse.bacc as bacc
    import concourse.tile as tile
    import concourse.mybir as mybir

    dt = mybir.dt
    nc = bacc.Bacc("TRN2", target_bir_lowering=False, debug=False,
                   num_devices=NCORES, dynamic_dma_scratch_size=32768,
                   num_swdge_queues=2)
    io = {}

    def din(name, shape, dtype):
        io[name] = nc.dram_tensor(name, shape, dtype, kind="ExternalInput").ap()

    din("f_all", [128, B * N], dt.bfloat16)
    din("w1t", [128, 128], dt.bfloat16)
    din("gamma", [128, 1], dt.float32)
    din("beta", [128, 1], dt.float32)
    din("pA", [24, N], dt.bfloat16)
    din("pB", [24, QPC], dt.bfloat16)
    din("tri_w", [128, NCHUNK + 4], dt.bfloat16)
    din("iota_j", [128, N], dt.int16)
    din("ident", [128, 128], dt.float32)
    din("dlp_s", [128, QPC * K], dt.bfloat16)
    io["out_o"] = nc.dram_tensor("out_o", [128, QPC], dt.float32,
                                 kind="ExternalOutput").ap()

    with tile.TileContext(nc) as tc:
        _kernel_body(tc, io)
    nc.compile()
    return nc


def _host_prep(inputs):
    p = np.asarray(inputs["p"], np.float32)
    f = np.asarray(inputs["f"], np.float32)
    dlp = np.asarray(inputs["dlp"], np.float32)
    W1 = np.asarray(inputs["W1"], np.float32)
    gamma = np.asarray(inputs["gamma"], np.float32)
    beta = np.asarray(inputs["beta"], np.float32)

    tri_w, iota_j, ident = _static_tables()
    f_cbn = np.moveaxis(f, 0, 1)                       # [C, B, N]
    w1t = np.ascontiguousarray(W1.T).astype(bf16)

    in_maps = []
    for core in range(NCORES):
        b, half = core // 2, core % 2
        q_lo, q_hi = half * QPC, (half + 1) * QPC
        pA, pB = _build_AB(p[b], q_lo, q_hi)
        dlp_s = np.ascontiguousarray(
            dlp[b, :, q_lo:q_hi, ::-1].reshape(C, QPC * K)).astype(bf16)
        # batch-rotated so this core's batch b sits in cols [0, N)
        f_all = np.ascontiguousarray(
            np.roll(f_cbn, -b, axis=1).reshape(C, B * N)).astype(bf16)
        in_maps.append({
            "f_all": f_all,
            "w1t": w1t,
            "gamma": np.ascontiguousarray(gamma.reshape(C, 1)),
            "beta": np.ascontiguousarray(beta.reshape(C, 1)),
            "pA": pA, "pB": pB,
            "tri_w": tri_w, "iota_j": iota_j, "ident": ident,
            "dlp_s": dlp_s,
        })
    return in_maps


def run(inputs, trace=False, **kw):
    from concourse.bass_utils import run_bass_kernel_spmd
    nc = _compiled()
    in_maps = _host_prep(inputs)
    res = run_bass_kernel_spmd(nc, in_maps, core_ids=list(range(NCORES)),
                               trace=trace, **kw)
    out = np.zeros((B, C, N), np.float32)
    for core in range(NCORES):
        b, half = core // 2, core % 2
        out[b, :, half * QPC:(half + 1) * QPC] = res.results[core]["out_o"]
    return out, res


def kernel(**inputs) -> np.ndarray:
    out, _ = run(inputs, trace=False)
    return out

